# revision 6
# baseline (speedup 1.0000x reference)
"""Trainium2 Bass kernel for MC-sampled cross-entropy-with-variance loss.

Computes mean over (s, b, h, w) of
    nll = logsumexp_c(mean + exp(0.5*log_var)*eps[s]) - logit[label]
distributed over 8 NeuronCores by sharding the H*W pixel axis.

v2 layout: classes (19) x 6 pixel-chunks packed on partitions, 2048
pixels per chunk on the free axis. eps is host-interleaved to
[b, chunk, class, sample, pixel] so each (image, region) needs ONE DMA
with 40 KB contiguous lines (10 samples per partition row) instead of
10 DMAs with 4 KB lines; mean/log_var ride one merged 8 KB-line DMA.
The label side uses host-staged index-selected views (eps/mean/log_var
at the label class), so the per-sample eps accumulation and one-hot
mask work disappear from the DVE. ln runs directly on the PSUM sumexp
with accum_out (no collect-buffer copy); exp and ln coexist in the
natural_log_exp_and_others ACT table set.
"""

import numpy as np
import ml_dtypes

import concourse.bass as bass
import concourse.bacc as bacc
import concourse.mybir as mybir
from concourse import tile
from concourse.bass_interp import get_hw_module
from concourse.bass_utils import run_bass_kernel_spmd
from concourse.mybir import ActivationFunctionType as Act

# ---------------------------------------------------------------- sizes
S, B, C, H, W = 10, 4, 19, 512, 512
HW = H * W
NCORES = 8
SLAB = HW // NCORES          # pixels per (core, b) = 32768
F = 2048                     # free-dim pixels per chunk
NJ = SLAB // F               # 16 chunks per slab
G_FULL = 6                   # chunks packed per full region (6*19=114 parts)
REGIONS = [(G_FULL, 0), (G_FULL, 6), (4, 12)]   # (num chunks, chunk offset)
MM_N = 512                   # matmul free-dim (PSUM bank limit)
F32 = mybir.dt.float32
BF16 = mybir.dt.bfloat16


def _combined_act_tables():
    """Restrict ACT table selection to natural_log_exp_and_others so the
    interleaved exp/ln activations share ONE table set (the default pass
    alternates exp_and_others/natural_log -> 24 x 2.7us reloads)."""
    import concourse.hw_specs as hw_specs

    orig = hw_specs.get_activation_tables

    def patched(arch):
        t = orig(arch)
        if "natural_log_exp_and_others" not in t:
            return t
        return {
            name: (funcs if name == "natural_log_exp_and_others" else set())
            for name, funcs in t.items()
        }

    return orig, patched


def build_program():
    import concourse.bacc as bacc_mod

    orig, patched = _combined_act_tables()
    bacc_mod.get_activation_tables = patched
    try:
        return _build_program_inner()
    finally:
        bacc_mod.get_activation_tables = orig


def _build_program_inner():
    nc = bacc.Bacc("TRN2", target_bir_lowering=False, debug=False,
                   num_devices=NCORES)

    # eps_il[b, j, c, s, x]: per-partition (j,c) line holds all 10
    # samples' pixels contiguously -> 40 KB DMA lines.
    eps_h = nc.dram_tensor("eps_il", [B, NJ, C, S, F], BF16,
                           kind="ExternalInput")
    # mlv_il[b, j, c, 0, x] = mean, [b, j, c, 1, x] = log_var
    mlv_h = nc.dram_tensor("mlv_il", [B, NJ, C, 2, F], BF16,
                           kind="ExternalInput")
    # label-selected views, packed [128, ...] with p = b*32 + x//1024
    eg_h = nc.dram_tensor("eg_il", [128, S, 1024], BF16,
                          kind="ExternalInput")
    mlvg_h = nc.dram_tensor("mlvg_il", [128, 2, 1024], BF16,
                            kind="ExternalInput")
    sel6_h = nc.dram_tensor("sel6", [S, 114, 64], BF16, kind="ExternalInput")
    sel4_h = nc.dram_tensor("sel4", [S, 76, 64], BF16, kind="ExternalInput")
    lse_h = nc.dram_tensor("lse_out", [60, 1], F32, kind="ExternalOutput")
    lab_o_h = nc.dram_tensor("lab_out", [128, 1], F32, kind="ExternalOutput")

    with tile.TileContext(nc) as tc:
        with (
            tc.tile_pool(name="consts", bufs=1) as consts,
            tc.tile_pool(name="epsp", bufs=7) as eps_pool,
            tc.tile_pool(name="region", bufs=2) as region_pool,
            tc.tile_pool(name="work", bufs=2) as work_pool,
            tc.tile_pool(name="accp", bufs=1) as acc_pool,
            tc.tile_pool(name="psum", bufs=2, space="PSUM") as psum_pool,
        ):
            sel6_sb, sel4_sb = [], []
            for s in range(S):
                t6 = consts.tile([114, 64], BF16, tag=f"sel6_{s}",
                                 name=f"sel6_{s}")
                nc.sync.dma_start(out=t6, in_=sel6_h.ap()[s])
                sel6_sb.append(t6)
                t4 = consts.tile([76, 64], BF16, tag=f"sel4_{s}",
                                 name=f"sel4_{s}")
                nc.sync.dma_start(out=t4, in_=sel4_h.ap()[s])
                sel4_sb.append(t4)

            acc_lse = acc_pool.tile([60, 1], F32)
            nc.vector.memset(acc_lse, 0.0)

            # ---------------- label side: host-staged gathered views
            egt = consts.tile([128, S * 1024], BF16, tag="egt")
            nc.sync.dma_start(out=egt, in_=bass.AP(
                tensor=eg_h, offset=0, ap=[[S * 1024, 128], [1, S * 1024]]))
            mlvgt = consts.tile([128, 2 * 1024], BF16, tag="mlvgt")
            nc.sync.dma_start(out=mlvgt, in_=bass.AP(
                tensor=mlvg_h, offset=0, ap=[[2 * 1024, 128], [1, 2 * 1024]]))
            stdg = consts.tile([128, 1024], BF16, tag="stdg")
            nc.scalar.activation(stdg, mlvgt[:, 1024:2048], Act.Exp,
                                 scale=0.5)
            egs = consts.tile([128, 1024], BF16, tag="egs")
            nc.vector.tensor_add(egs, egt[:, 0:1024], egt[:, 1024:2048])
            for s in range(2, S):
                nc.vector.tensor_add(
                    egs, egs, egt[:, s * 1024:(s + 1) * 1024])
            lgt = consts.tile([128, 1024], BF16, tag="lgt")
            nc.vector.tensor_mul(lgt, egs, stdg)
            lab_p = acc_pool.tile([128, 1], F32)
            lgu = consts.tile([128, 1024], BF16, tag="lgu")
            # lgu = 10*mean_g + std_g*eps_sum_g, summed over pixels
            nc.vector.scalar_tensor_tensor(
                lgu, mlvgt[:, 0:1024], 10.0, lgt,
                mybir.AluOpType.mult, mybir.AluOpType.add,
                accum_out=lab_p,
            )
            nc.sync.dma_start(out=lab_o_h.ap(), in_=lab_p)

            # ---------------- main loop over (image, region)
            for b in range(B):
                for r, (g, j0) in enumerate(REGIONS):
                    p_ = g * C          # active partitions (114 or 76)
                    rows = g * S        # psum rows used (60 or 40)
                    sel_sb = sel6_sb if g == G_FULL else sel4_sb

                    # one DMA per sample-pair (8 KB lines), alternating
                    # HWDGE rings so descriptors spread across more SDMA
                    # engines (a single big DMA lands on only ~4 of 16)
                    epts = []
                    for sp in range(S // 2):
                        et = eps_pool.tile([114, 2 * F], BF16, tag="ept")
                        eng = nc.sync if sp % 2 == 0 else nc.scalar
                        eng.dma_start(
                            out=et[:p_, :],
                            in_=bass.AP(
                                tensor=eps_h,
                                offset=(b * NJ + j0) * C * S * F
                                + 2 * sp * F,
                                ap=[[C * S * F, g], [S * F, C], [1, 2 * F]],
                            ),
                        )
                        epts.append(et)
                    mlvt = region_pool.tile([114, 2 * F], BF16, tag="mlv")
                    nc.scalar.dma_start(
                        out=mlvt[:p_, :],
                        in_=bass.AP(
                            tensor=mlv_h,
                            offset=(b * NJ + j0) * C * 2 * F,
                            ap=[[C * 2 * F, g], [2 * F, C], [1, 2 * F]],
                        ),
                    )
                    mt = mlvt[:, 0:F]
                    std = region_pool.tile([114, F], BF16, tag="std")
                    nc.scalar.activation(std[:p_], mlvt[:p_, F:2 * F],
                                         Act.Exp, scale=0.5)

                    psum_t = psum_pool.tile([64, F], F32, tag="psum")

                    for sp in range(S // 2):
                        t2 = work_pool.tile([114, 2 * F], BF16, tag="t2p")
                        for h in range(2):
                            s = 2 * sp + h
                            half = t2[:, h * F:(h + 1) * F]
                            nc.vector.tensor_mul(
                                half[:p_],
                                epts[sp][:p_, h * F:(h + 1) * F],
                                std[:p_],
                            )
                            nc.vector.tensor_add(
                                half[:p_], half[:p_], mt[:p_],
                            )
                        e1 = work_pool.tile([114, 2 * F], BF16, tag="e1p")
                        nc.scalar.activation(e1[:p_], t2[:p_], Act.Exp)
                        for k in range(2 * F // MM_N):
                            s_idx = 2 * sp + (k * MM_N) // F
                            nc.tensor.matmul(
                                psum_t[:, (k * MM_N) % F:
                                       (k * MM_N) % F + MM_N],
                                sel_sb[s_idx],
                                e1[:p_, k * MM_N:(k + 1) * MM_N],
                                start=(sp == 0 and k < F // MM_N),
                                stop=(sp == S // 2 - 1 and k >= F // MM_N),
                            )

                    # ln directly from PSUM; accum_out sums over pixels
                    lnb = work_pool.tile([64, F], BF16, tag="lnb")
                    lse_p = work_pool.tile([60, 1], F32, tag="lsep")
                    nc.scalar.activation(lnb[:rows], psum_t[:rows], Act.Ln,
                                         accum_out=lse_p[:rows])
                    nc.vector.tensor_add(acc_lse[:rows], acc_lse[:rows],
                                         lse_p[:rows])

            nc.sync.dma_start(out=lse_h.ap(), in_=acc_lse)

    nc.compile()
    nc.m = get_hw_module(nc.m)
    return nc


def _sels():
    # partition p = j * 19 + c  (chunk-outer, class-inner)
    sel6 = np.zeros((S, 114, 64), dtype=ml_dtypes.bfloat16)
    sel4 = np.zeros((S, 76, 64), dtype=ml_dtypes.bfloat16)
    for s in range(S):
        for p in range(114):
            sel6[s, p, 6 * s + p // C] = 1.0
        for p in range(76):
            sel4[s, p, 4 * s + p // C] = 1.0
    return sel6, sel4


def kernel(mean, log_var, label, eps, _trace=False):
    mean = np.asarray(mean, dtype=np.float32).reshape(B, C, HW)
    log_var = np.asarray(log_var, dtype=np.float32).reshape(B, C, HW)
    label_i = np.asarray(label).reshape(B, HW).astype(np.int64)
    eps_r = np.asarray(eps, dtype=np.float32).reshape(S, B, C, HW)

    # label-gathered views (index staging; arithmetic stays on device)
    bi = np.arange(B)[:, None]
    mg = mean[bi, label_i, np.arange(HW)[None, :]]          # [B, HW]
    lvg = log_var[bi, label_i, np.arange(HW)[None, :]]      # [B, HW]
    eg = eps_r[:, bi, label_i, np.arange(HW)[None, :]]      # [S, B, HW]

    sel6, sel4 = _sels()
    in_maps = []
    for c in range(NCORES):
        lo, hi = c * SLAB, (c + 1) * SLAB
        # [S,B,C,slab] -> [B, j, C, S, F]
        e_il = np.ascontiguousarray(
            eps_r[:, :, :, lo:hi].reshape(S, B, C, NJ, F)
            .transpose(1, 3, 2, 0, 4)).astype(ml_dtypes.bfloat16)
        mlv = np.stack([mean[:, :, lo:hi], log_var[:, :, lo:hi]], axis=2)
        # [B, C, 2, slab] -> [B, j, C, 2, F]
        mlv_il = np.ascontiguousarray(
            mlv.reshape(B, C, 2, NJ, F).transpose(0, 3, 1, 2, 4)
        ).astype(ml_dtypes.bfloat16)
        # [S, B, slab] -> [p=b*32+x//1024, S, 1024]
        eg_il = np.ascontiguousarray(
            eg[:, :, lo:hi].reshape(S, B * 32, 1024).transpose(1, 0, 2)
        ).astype(ml_dtypes.bfloat16)
        mlvg = np.stack([mg[:, lo:hi], lvg[:, lo:hi]], axis=1)  # [B,2,slab]
        mlvg_il = np.ascontiguousarray(
            mlvg.reshape(B, 2, 32, 1024).transpose(0, 2, 1, 3)
            .reshape(128, 2, 1024)).astype(ml_dtypes.bfloat16)
        in_maps.append({
            "eps_il": e_il,
            "mlv_il": mlv_il,
            "eg_il": eg_il,
            "mlvg_il": mlvg_il,
            "sel6": sel6,
            "sel4": sel4,
        })

    nc = build_program()
    res = run_bass_kernel_spmd(
        nc, in_maps, core_ids=list(range(NCORES)), trace=_trace
    )
    global last_results
    last_results = res

    total = np.float64(0.0)
    for c in range(NCORES):
        total += res.results[c]["lse_out"].astype(np.float64).sum()
        total -= res.results[c]["lab_out"].astype(np.float64).sum()
    loss = total / float(S * B * HW)
    return np.float32(loss)


# revision 9
# speedup vs baseline: 1.1348x; 1.1348x over previous
"""Trainium2 Bass kernel for MC-sampled cross-entropy-with-variance loss.

Computes mean over (s, b, h, w) of
    nll = logsumexp_c(mean + exp(0.5*log_var)*eps[s]) - logit[label]
distributed over 8 NeuronCores by sharding the H*W pixel axis.

v3 layout: every region is a FULL 6-chunk pack (19 classes x 6 chunks =
114 partitions); region pixel widths are (2048, 2048, 1366) so the
16384+8196-pixel slab is covered with only 4 pad pixels per image
instead of a 4-chunk tail region idling a third of the partitions.
The pad contributes exactly ln(19) per (sample, pad pixel) and is
subtracted analytically on the host. eps is host-interleaved to
[b, chunk, class, sample, pixel] so each (image, region, sample-pair)
is one DMA with 2*F-byte-contiguous lines, alternating the two HWDGE
rings; mean/log_var ride one merged DMA per region. The label side
uses host-staged index-selected views (pure gather; all arithmetic on
device). ln runs directly on the PSUM sumexp with accum_out, and exp/ln
share the natural_log_exp_and_others ACT table set (one load).
"""

import numpy as np
import ml_dtypes

import concourse.bass as bass
import concourse.bacc as bacc
import concourse.mybir as mybir
from concourse import tile
from concourse.bass_interp import get_hw_module
from concourse.bass_utils import run_bass_kernel_spmd
from concourse.mybir import ActivationFunctionType as Act

# ---------------------------------------------------------------- sizes
S, B, C, H, W = 10, 4, 19, 512, 512
HW = H * W
NCORES = 8
SLAB = HW // NCORES          # pixels per (core, b) = 32768
G = 6                        # chunks per region (G*C = 114 partitions)
FS = [2048, 2048, 1366]      # pixels per chunk, per region
OFFS = [0, 12288, 24576]     # region pixel offsets (6*F each)
PAD = 4                      # 24576 + 6*1366 = 32772 = SLAB + 4
MM_N = 512                   # matmul free-dim chunk (PSUM bank limit)
F32 = mybir.dt.float32
BF16 = mybir.dt.bfloat16


def _combined_act_tables():
    """Restrict ACT table selection to natural_log_exp_and_others so the
    interleaved exp/ln activations share ONE table set (the default pass
    alternates exp_and_others/natural_log -> 24 x 2.7us reloads)."""
    import concourse.hw_specs as hw_specs

    orig = hw_specs.get_activation_tables

    def patched(arch):
        t = orig(arch)
        if "natural_log_exp_and_others" not in t:
            return t
        return {
            name: (funcs if name == "natural_log_exp_and_others" else set())
            for name, funcs in t.items()
        }

    return orig, patched


def build_program():
    import concourse.bacc as bacc_mod

    orig, patched = _combined_act_tables()
    bacc_mod.get_activation_tables = patched
    try:
        return _build_program_inner()
    finally:
        bacc_mod.get_activation_tables = orig


def _build_program_inner():
    nc = bacc.Bacc("TRN2", target_bir_lowering=False, debug=False,
                   num_devices=NCORES)

    # eps_r{k}[b, j, c, s, x]: per-partition (j,c) line holds all 10
    # samples' pixels contiguously; one DMA per sample-pair -> 2*F lines
    eps_h = [
        nc.dram_tensor(f"eps_r{k}", [B, G, C, S, FS[k]], BF16,
                       kind="ExternalInput")
        for k in range(3)
    ]
    # mlv_r{k}[b, j, c, 0, x] = mean, [b, j, c, 1, x] = log_var
    mlv_h = [
        nc.dram_tensor(f"mlv_r{k}", [B, G, C, 2, FS[k]], BF16,
                       kind="ExternalInput")
        for k in range(3)
    ]
    # label-selected views, packed [128, ...] with p = b*32 + x//1024
    eg_h = nc.dram_tensor("eg_il", [128, S, 1024], BF16,
                          kind="ExternalInput")
    mlvg_h = nc.dram_tensor("mlvg_il", [128, 2, 1024], BF16,
                            kind="ExternalInput")
    sel6_h = nc.dram_tensor("sel6", [S, 114, 64], BF16, kind="ExternalInput")
    lse_h = nc.dram_tensor("lse_out", [60, 1], F32, kind="ExternalOutput")
    lab_o_h = nc.dram_tensor("lab_out", [128, 1], F32, kind="ExternalOutput")

    with tile.TileContext(nc) as tc:
        with (
            tc.tile_pool(name="consts", bufs=1) as consts,
            tc.tile_pool(name="epsp", bufs=7) as eps_pool,
            tc.tile_pool(name="region", bufs=2) as region_pool,
            tc.tile_pool(name="work", bufs=2) as work_pool,
            tc.tile_pool(name="accp", bufs=1) as acc_pool,
            tc.tile_pool(name="psum", bufs=2, space="PSUM") as psum_pool,
        ):
            sel_sb = []
            for s in range(S):
                t6 = consts.tile([114, 64], BF16, tag=f"sel6_{s}",
                                 name=f"sel6_{s}")
                nc.sync.dma_start(out=t6, in_=sel6_h.ap()[s])
                sel_sb.append(t6)

            acc_lse = acc_pool.tile([60, 1], F32)
            nc.vector.memset(acc_lse, 0.0)

            # ---------------- label side: host-staged gathered views
            egt = consts.tile([128, S * 1024], BF16, tag="egt")
            nc.scalar.dma_start(out=egt, in_=bass.AP(
                tensor=eg_h, offset=0, ap=[[S * 1024, 128], [1, S * 1024]]))
            mlvgt = consts.tile([128, 2 * 1024], BF16, tag="mlvgt")
            nc.scalar.dma_start(out=mlvgt, in_=bass.AP(
                tensor=mlvg_h, offset=0, ap=[[2 * 1024, 128], [1, 2 * 1024]]))
            stdg = consts.tile([128, 1024], BF16, tag="stdg")
            nc.scalar.activation(stdg, mlvgt[:, 1024:2048], Act.Exp,
                                 scale=0.5)
            egs = consts.tile([128, 1024], BF16, tag="egs")
            nc.vector.tensor_add(egs, egt[:, 0:1024], egt[:, 1024:2048])
            for s in range(2, S):
                nc.vector.tensor_add(
                    egs, egs, egt[:, s * 1024:(s + 1) * 1024])
            lgt = consts.tile([128, 1024], BF16, tag="lgt")
            nc.vector.tensor_mul(lgt, egs, stdg)
            lab_p = acc_pool.tile([128, 1], F32)
            lgu = consts.tile([128, 1024], BF16, tag="lgu")
            # lgu = 10*mean_g + std_g*eps_sum_g, summed over pixels
            nc.vector.scalar_tensor_tensor(
                lgu, mlvgt[:, 0:1024], 10.0, lgt,
                mybir.AluOpType.mult, mybir.AluOpType.add,
                accum_out=lab_p,
            )
            nc.sync.dma_start(out=lab_o_h.ap(), in_=lab_p)

            # ---------------- main loop over (image, region)
            for b in range(B):
                for r in range(3):
                    f = FS[r]
                    # one DMA per sample-pair, alternating HWDGE rings so
                    # lines spread across more SDMA engines
                    epts = []
                    for sp in range(S // 2):
                        et = eps_pool.tile([114, 2 * FS[0]], BF16,
                                           tag="ept")
                        eng = nc.sync if sp % 2 == 0 else nc.scalar
                        # single-level partition ladder: lines round-robin
                        # across all 16 SDMA engines (a 2-level ladder
                        # pins each outer iteration to one engine)
                        eng.dma_start(
                            out=et[:, :2 * f],
                            in_=bass.AP(
                                tensor=eps_h[r],
                                offset=b * G * C * S * f + 2 * sp * f,
                                ap=[[S * f, G * C], [1, 2 * f]],
                            ),
                        )
                        epts.append(et)
                    mlvt = region_pool.tile([114, 2 * FS[0]], BF16,
                                            tag="mlv")
                    nc.scalar.dma_start(
                        out=mlvt[:, :2 * f],
                        in_=bass.AP(
                            tensor=mlv_h[r],
                            offset=b * G * C * 2 * f,
                            ap=[[2 * f, G * C], [1, 2 * f]],
                        ),
                    )
                    mt = mlvt[:, 0:f]
                    std = region_pool.tile([114, FS[0]], BF16, tag="std")
                    nc.scalar.activation(std[:, :f], mlvt[:, f:2 * f],
                                         Act.Exp, scale=0.5)

                    psum_t = psum_pool.tile([64, FS[0]], F32, tag="psum")

                    for sp in range(S // 2):
                        t2 = work_pool.tile([114, 2 * FS[0]], BF16,
                                            tag="t2p")
                        for h in range(2):
                            half = t2[:, h * f:(h + 1) * f]
                            nc.vector.tensor_mul(
                                half, epts[sp][:, h * f:(h + 1) * f],
                                std[:, :f],
                            )
                            nc.vector.tensor_add(half, half, mt)
                        e1 = work_pool.tile([114, 2 * FS[0]], BF16,
                                            tag="e1p")
                        nc.scalar.activation(e1[:, :2 * f], t2[:, :2 * f],
                                             Act.Exp)
                        # class-sum matmuls: free-dim chunks of <=512
                        nk = (f + MM_N - 1) // MM_N
                        for h in range(2):
                            s = 2 * sp + h
                            for k in range(nk):
                                c0 = k * MM_N
                                c1 = min(f, c0 + MM_N)
                                nc.tensor.matmul(
                                    psum_t[:, c0:c1],
                                    sel_sb[s],
                                    e1[:, h * f + c0:h * f + c1],
                                    start=(sp == 0 and h == 0),
                                    stop=(sp == S // 2 - 1 and h == 1),
                                )

                    # ln directly from PSUM; accum_out sums over pixels
                    lnb = work_pool.tile([64, FS[0]], BF16, tag="lnb")
                    lse_p = work_pool.tile([60, 1], F32, tag="lsep")
                    nc.scalar.activation(lnb[:60, :f], psum_t[:60, :f],
                                         Act.Ln, accum_out=lse_p)
                    nc.vector.tensor_add(acc_lse, acc_lse, lse_p)

            nc.sync.dma_start(out=lse_h.ap(), in_=acc_lse)

    nc.compile()
    nc.m = get_hw_module(nc.m)
    return nc


def _sels():
    # partition p = j * 19 + c  (chunk-outer, class-inner)
    sel6 = np.zeros((S, 114, 64), dtype=ml_dtypes.bfloat16)
    for s in range(S):
        for p in range(114):
            sel6[s, p, 6 * s + p // C] = 1.0
    return sel6


def kernel(mean, log_var, label, eps, _trace=False):
    mean = np.asarray(mean, dtype=np.float32).reshape(B, C, HW)
    log_var = np.asarray(log_var, dtype=np.float32).reshape(B, C, HW)
    label_i = np.asarray(label).reshape(B, HW).astype(np.int64)
    eps_r = np.asarray(eps, dtype=np.float32).reshape(S, B, C, HW)

    # label-gathered views (index staging; arithmetic stays on device)
    bi = np.arange(B)[:, None]
    ci = np.arange(HW)[None, :]
    mg = mean[bi, label_i, ci]                               # [B, HW]
    lvg = log_var[bi, label_i, ci]                           # [B, HW]
    eg = eps_r[:, bi, label_i, ci]                           # [S, B, HW]

    sel6 = _sels()
    in_maps = []
    for c in range(NCORES):
        lo, hi = c * SLAB, (c + 1) * SLAB
        im = {"sel6": sel6}
        for k in range(3):
            f = FS[k]
            a, z = lo + OFFS[k], lo + OFFS[k] + G * f
            if z <= hi:
                esl = eps_r[:, :, :, a:z]
                msl = mean[:, :, a:z]
                lsl = log_var[:, :, a:z]
            else:  # pad the 4-pixel tail of region 2 with zeros
                pw = z - hi
                esl = np.concatenate(
                    [eps_r[:, :, :, a:hi],
                     np.zeros((S, B, C, pw), np.float32)], axis=3)
                msl = np.concatenate(
                    [mean[:, :, a:hi], np.zeros((B, C, pw), np.float32)],
                    axis=2)
                lsl = np.concatenate(
                    [log_var[:, :, a:hi], np.zeros((B, C, pw), np.float32)],
                    axis=2)
            im[f"eps_r{k}"] = np.ascontiguousarray(
                esl.reshape(S, B, C, G, f).transpose(1, 3, 2, 0, 4)
            ).astype(ml_dtypes.bfloat16)
            im[f"mlv_r{k}"] = np.ascontiguousarray(
                np.stack([msl, lsl], axis=2).reshape(B, C, 2, G, f)
                .transpose(0, 3, 1, 2, 4)).astype(ml_dtypes.bfloat16)
        im["eg_il"] = np.ascontiguousarray(
            eg[:, :, lo:hi].reshape(S, B * 32, 1024).transpose(1, 0, 2)
        ).astype(ml_dtypes.bfloat16)
        im["mlvg_il"] = np.ascontiguousarray(
            np.stack([mg[:, lo:hi], lvg[:, lo:hi]], axis=1)
            .reshape(B, 2, 32, 1024).transpose(0, 2, 1, 3)
            .reshape(128, 2, 1024)).astype(ml_dtypes.bfloat16)
        in_maps.append(im)

    nc = build_program()
    res = run_bass_kernel_spmd(
        nc, in_maps, core_ids=list(range(NCORES)), trace=_trace
    )
    global last_results
    last_results = res

    total = np.float64(0.0)
    for c in range(NCORES):
        total += res.results[c]["lse_out"].astype(np.float64).sum()
        total -= res.results[c]["lab_out"].astype(np.float64).sum()
    # remove the analytic contribution of the PAD pixels:
    # each pad pixel adds ln(19) per (core, image, sample)
    total -= np.float64(NCORES * B * S * PAD) * np.log(np.float64(C))
    loss = total / float(S * B * HW)
    return np.float32(loss)


# revision 11
# speedup vs baseline: 1.2495x; 1.1011x over previous
"""Trainium2 Bass kernel for MC-sampled cross-entropy-with-variance loss.

Computes mean over (s, b, h, w) of
    nll = logsumexp_c(mean + exp(0.5*log_var)*eps[s]) - logit[label]
distributed over 8 NeuronCores by sharding the H*W pixel axis.

v3 layout: every region is a FULL 6-chunk pack (19 classes x 6 chunks =
114 partitions); region pixel widths are (2048, 2048, 1366) so the
16384+8196-pixel slab is covered with only 4 pad pixels per image
instead of a 4-chunk tail region idling a third of the partitions.
The pad contributes exactly ln(19) per (sample, pad pixel) and is
subtracted analytically on the host. eps is host-interleaved to
[b, chunk, class, sample, pixel] so each (image, region, sample-pair)
is one DMA with 2*F-byte-contiguous lines, alternating the two HWDGE
rings; mean/log_var ride one merged DMA per region. The label side
uses host-staged index-selected views (pure gather; all arithmetic on
device). ln runs directly on the PSUM sumexp with accum_out, and exp/ln
share the natural_log_exp_and_others ACT table set (one load).
"""

import numpy as np
import ml_dtypes

import concourse.bass as bass
import concourse.bacc as bacc
import concourse.mybir as mybir
from concourse import tile
from concourse.bass_interp import get_hw_module
from concourse.bass_utils import run_bass_kernel_spmd
from concourse.mybir import ActivationFunctionType as Act

# ---------------------------------------------------------------- sizes
S, B, C, H, W = 10, 4, 19, 512, 512
HW = H * W
NCORES = 8
SLAB = HW // NCORES          # pixels per (core, b) = 32768
G = 6                        # chunks per region (G*C = 114 partitions)
FS = [2048, 2048, 1366]      # pixels per chunk, per region
OFFS = [0, 12288, 24576]     # region pixel offsets (6*F each)
PAD = 4                      # 24576 + 6*1366 = 32772 = SLAB + 4
MM_N = 512                   # matmul free-dim chunk (PSUM bank limit)
F32 = mybir.dt.float32
BF16 = mybir.dt.bfloat16


def _combined_act_tables():
    """Restrict ACT table selection to natural_log_exp_and_others so the
    interleaved exp/ln activations share ONE table set (the default pass
    alternates exp_and_others/natural_log -> 24 x 2.7us reloads)."""
    import concourse.hw_specs as hw_specs

    orig = hw_specs.get_activation_tables

    def patched(arch):
        t = orig(arch)
        if "natural_log_exp_and_others" not in t:
            return t
        return {
            name: (funcs if name == "natural_log_exp_and_others" else set())
            for name, funcs in t.items()
        }

    return orig, patched


def build_program():
    import concourse.bacc as bacc_mod

    orig, patched = _combined_act_tables()
    bacc_mod.get_activation_tables = patched
    try:
        return _build_program_inner()
    finally:
        bacc_mod.get_activation_tables = orig


def _build_program_inner():
    nc = bacc.Bacc("TRN2", target_bir_lowering=False, debug=False,
                   num_devices=NCORES)

    # eps_r{k}[b, j, c, s, x]: per-partition (j,c) line holds all 10
    # samples' pixels contiguously; one DMA per sample-pair -> 2*F lines
    eps_h = [
        nc.dram_tensor(f"eps_r{k}", [B, G, C, S, FS[k]], BF16,
                       kind="ExternalInput")
        for k in range(3)
    ]
    # mlv_r{k}[b, j, c, 0, x] = mean, [b, j, c, 1, x] = log_var
    mlv_h = [
        nc.dram_tensor(f"mlv_r{k}", [B, G, C, 2, FS[k]], BF16,
                       kind="ExternalInput")
        for k in range(3)
    ]
    # label-selected views, packed [128, ...] with p = b*32 + x//1024
    eg_h = nc.dram_tensor("eg_il", [128, S, 1024], BF16,
                          kind="ExternalInput")
    mlvg_h = nc.dram_tensor("mlvg_il", [128, 2, 1024], BF16,
                            kind="ExternalInput")
    sel6_h = nc.dram_tensor("sel6", [S, 114, 64], BF16, kind="ExternalInput")
    lse_h = nc.dram_tensor("lse_out", [60, 1], F32, kind="ExternalOutput")
    lab_o_h = nc.dram_tensor("lab_out", [128, 1], F32, kind="ExternalOutput")

    with tile.TileContext(nc) as tc:
        with (
            tc.tile_pool(name="consts", bufs=1) as consts,
            tc.tile_pool(name="epsp", bufs=7) as eps_pool,
            tc.tile_pool(name="region", bufs=2) as region_pool,
            tc.tile_pool(name="work", bufs=2) as work_pool,
            tc.tile_pool(name="accp", bufs=1) as acc_pool,
            tc.tile_pool(name="psum", bufs=2, space="PSUM") as psum_pool,
        ):
            sel_sb = []
            for s in range(S):
                t6 = consts.tile([114, 64], BF16, tag=f"sel6_{s}",
                                 name=f"sel6_{s}")
                nc.sync.dma_start(out=t6, in_=sel6_h.ap()[s])
                sel_sb.append(t6)

            acc_lse = acc_pool.tile([60, 1], F32)
            nc.vector.memset(acc_lse, 0.0)

            # ---------------- label side: host-staged gathered views
            egt = consts.tile([128, S * 1024], BF16, tag="egt")
            nc.scalar.dma_start(out=egt, in_=bass.AP(
                tensor=eg_h, offset=0, ap=[[S * 1024, 128], [1, S * 1024]]))
            mlvgt = consts.tile([128, 2 * 1024], BF16, tag="mlvgt")
            nc.scalar.dma_start(out=mlvgt, in_=bass.AP(
                tensor=mlvg_h, offset=0, ap=[[2 * 1024, 128], [1, 2 * 1024]]))
            stdg = consts.tile([128, 1024], BF16, tag="stdg")
            nc.scalar.activation(stdg, mlvgt[:, 1024:2048], Act.Exp,
                                 scale=0.5)
            egs = consts.tile([128, 1024], BF16, tag="egs")
            nc.vector.tensor_add(egs, egt[:, 0:1024], egt[:, 1024:2048])
            for s in range(2, S):
                nc.vector.tensor_add(
                    egs, egs, egt[:, s * 1024:(s + 1) * 1024])
            lgt = consts.tile([128, 1024], BF16, tag="lgt")
            nc.vector.tensor_mul(lgt, egs, stdg)
            lab_p = acc_pool.tile([128, 1], F32)
            lgu = consts.tile([128, 1024], BF16, tag="lgu")
            # lgu = 10*mean_g + std_g*eps_sum_g, summed over pixels
            nc.vector.scalar_tensor_tensor(
                lgu, mlvgt[:, 0:1024], 10.0, lgt,
                mybir.AluOpType.mult, mybir.AluOpType.add,
                accum_out=lab_p,
            )
            nc.sync.dma_start(out=lab_o_h.ap(), in_=lab_p)

            # ---------------- main loop over (image, region)
            for b in range(B):
                for r in range(3):
                    f = FS[r]
                    # one DMA per sample-pair, alternating HWDGE rings so
                    # lines spread across more SDMA engines
                    epts = []
                    for sp in range(S // 2):
                        et = eps_pool.tile([114, 2 * FS[0]], BF16,
                                           tag="ept")
                        eng = nc.sync if sp % 2 == 0 else nc.scalar
                        # the DGE splits a DMA's lines evenly across the
                        # largest SDMA-engine count dividing the partition
                        # count: 114 -> 6 engines, 112 -> all 16. Issue as
                        # 112 + 2 partitions to engage the full pool.
                        off = b * G * C * S * f + 2 * sp * f
                        eng.dma_start(
                            out=et[:112, :2 * f],
                            in_=bass.AP(
                                tensor=eps_h[r],
                                offset=off,
                                ap=[[S * f, 112], [1, 2 * f]],
                            ),
                        )
                        eng.dma_start(
                            out=et[112:114, :2 * f],
                            in_=bass.AP(
                                tensor=eps_h[r],
                                offset=off + 112 * S * f,
                                ap=[[S * f, 2], [1, 2 * f]],
                            ),
                        )
                        epts.append(et)
                    mlvt = region_pool.tile([114, 2 * FS[0]], BF16,
                                            tag="mlv")
                    moff = b * G * C * 2 * f
                    nc.scalar.dma_start(
                        out=mlvt[:112, :2 * f],
                        in_=bass.AP(
                            tensor=mlv_h[r],
                            offset=moff,
                            ap=[[2 * f, 112], [1, 2 * f]],
                        ),
                    )
                    nc.scalar.dma_start(
                        out=mlvt[112:114, :2 * f],
                        in_=bass.AP(
                            tensor=mlv_h[r],
                            offset=moff + 112 * 2 * f,
                            ap=[[2 * f, 2], [1, 2 * f]],
                        ),
                    )
                    mt = mlvt[:, 0:f]
                    std = region_pool.tile([114, FS[0]], BF16, tag="std")
                    nc.scalar.activation(std[:, :f], mlvt[:, f:2 * f],
                                         Act.Exp, scale=0.5)

                    psum_t = psum_pool.tile([64, FS[0]], F32, tag="psum")

                    for sp in range(S // 2):
                        t2 = work_pool.tile([114, 2 * FS[0]], BF16,
                                            tag="t2p")
                        for h in range(2):
                            half = t2[:, h * f:(h + 1) * f]
                            nc.vector.tensor_mul(
                                half, epts[sp][:, h * f:(h + 1) * f],
                                std[:, :f],
                            )
                            nc.vector.tensor_add(half, half, mt)
                        e1 = work_pool.tile([114, 2 * FS[0]], BF16,
                                            tag="e1p")
                        nc.scalar.activation(e1[:, :2 * f], t2[:, :2 * f],
                                             Act.Exp)
                        # class-sum matmuls: free-dim chunks of <=512
                        nk = (f + MM_N - 1) // MM_N
                        for h in range(2):
                            s = 2 * sp + h
                            for k in range(nk):
                                c0 = k * MM_N
                                c1 = min(f, c0 + MM_N)
                                nc.tensor.matmul(
                                    psum_t[:, c0:c1],
                                    sel_sb[s],
                                    e1[:, h * f + c0:h * f + c1],
                                    start=(sp == 0 and h == 0),
                                    stop=(sp == S // 2 - 1 and h == 1),
                                )

                    # ln directly from PSUM; accum_out sums over pixels
                    lnb = work_pool.tile([64, FS[0]], BF16, tag="lnb")
                    lse_p = work_pool.tile([60, 1], F32, tag="lsep")
                    nc.scalar.activation(lnb[:60, :f], psum_t[:60, :f],
                                         Act.Ln, accum_out=lse_p)
                    nc.vector.tensor_add(acc_lse, acc_lse, lse_p)

            nc.sync.dma_start(out=lse_h.ap(), in_=acc_lse)

    nc.compile()
    nc.m = get_hw_module(nc.m)
    return nc


def _sels():
    # partition p = j * 19 + c  (chunk-outer, class-inner)
    sel6 = np.zeros((S, 114, 64), dtype=ml_dtypes.bfloat16)
    for s in range(S):
        for p in range(114):
            sel6[s, p, 6 * s + p // C] = 1.0
    return sel6


def kernel(mean, log_var, label, eps, _trace=False):
    mean = np.asarray(mean, dtype=np.float32).reshape(B, C, HW)
    log_var = np.asarray(log_var, dtype=np.float32).reshape(B, C, HW)
    label_i = np.asarray(label).reshape(B, HW).astype(np.int64)
    eps_r = np.asarray(eps, dtype=np.float32).reshape(S, B, C, HW)

    # label-gathered views (index staging; arithmetic stays on device)
    bi = np.arange(B)[:, None]
    ci = np.arange(HW)[None, :]
    mg = mean[bi, label_i, ci]                               # [B, HW]
    lvg = log_var[bi, label_i, ci]                           # [B, HW]
    eg = eps_r[:, bi, label_i, ci]                           # [S, B, HW]

    sel6 = _sels()
    in_maps = []
    for c in range(NCORES):
        lo, hi = c * SLAB, (c + 1) * SLAB
        im = {"sel6": sel6}
        for k in range(3):
            f = FS[k]
            a, z = lo + OFFS[k], lo + OFFS[k] + G * f
            if z <= hi:
                esl = eps_r[:, :, :, a:z]
                msl = mean[:, :, a:z]
                lsl = log_var[:, :, a:z]
            else:  # pad the 4-pixel tail of region 2 with zeros
                pw = z - hi
                esl = np.concatenate(
                    [eps_r[:, :, :, a:hi],
                     np.zeros((S, B, C, pw), np.float32)], axis=3)
                msl = np.concatenate(
                    [mean[:, :, a:hi], np.zeros((B, C, pw), np.float32)],
                    axis=2)
                lsl = np.concatenate(
                    [log_var[:, :, a:hi], np.zeros((B, C, pw), np.float32)],
                    axis=2)
            im[f"eps_r{k}"] = np.ascontiguousarray(
                esl.reshape(S, B, C, G, f).transpose(1, 3, 2, 0, 4)
            ).astype(ml_dtypes.bfloat16)
            im[f"mlv_r{k}"] = np.ascontiguousarray(
                np.stack([msl, lsl], axis=2).reshape(B, C, 2, G, f)
                .transpose(0, 3, 1, 2, 4)).astype(ml_dtypes.bfloat16)
        im["eg_il"] = np.ascontiguousarray(
            eg[:, :, lo:hi].reshape(S, B * 32, 1024).transpose(1, 0, 2)
        ).astype(ml_dtypes.bfloat16)
        im["mlvg_il"] = np.ascontiguousarray(
            np.stack([mg[:, lo:hi], lvg[:, lo:hi]], axis=1)
            .reshape(B, 2, 32, 1024).transpose(0, 2, 1, 3)
            .reshape(128, 2, 1024)).astype(ml_dtypes.bfloat16)
        in_maps.append(im)

    nc = build_program()
    res = run_bass_kernel_spmd(
        nc, in_maps, core_ids=list(range(NCORES)), trace=_trace
    )
    global last_results
    last_results = res

    total = np.float64(0.0)
    for c in range(NCORES):
        total += res.results[c]["lse_out"].astype(np.float64).sum()
        total -= res.results[c]["lab_out"].astype(np.float64).sum()
    # remove the analytic contribution of the PAD pixels:
    # each pad pixel adds ln(19) per (core, image, sample)
    total -= np.float64(NCORES * B * S * PAD) * np.log(np.float64(C))
    loss = total / float(S * B * HW)
    return np.float32(loss)


# revision 19
# speedup vs baseline: 1.2541x; 1.0036x over previous
"""Trainium2 Bass kernel for MC-sampled cross-entropy-with-variance loss.

Computes mean over (s, b, h, w) of
    nll = logsumexp_c(mean + exp(0.5*log_var)*eps[s]) - logit[label]
distributed over 8 NeuronCores by sharding the H*W pixel axis.

v3 layout: every region is a FULL 6-chunk pack (19 classes x 6 chunks =
114 partitions); region pixel widths are (2048, 2048, 1366) so the
16384+8196-pixel slab is covered with only 4 pad pixels per image
instead of a 4-chunk tail region idling a third of the partitions.
The pad contributes exactly ln(19) per (sample, pad pixel) and is
subtracted analytically on the host. eps is host-interleaved to
[b, chunk, class, sample, pixel] so each (image, region, sample-pair)
is one DMA with 2*F-byte-contiguous lines, alternating the two HWDGE
rings; mean/log_var ride one merged DMA per region. The label side
uses host-staged index-selected views (pure gather; all arithmetic on
device). ln runs directly on the PSUM sumexp with accum_out, and exp/ln
share the natural_log_exp_and_others ACT table set (one load).
"""

import numpy as np
import ml_dtypes

import concourse.bass as bass
import concourse.bacc as bacc
import concourse.mybir as mybir
from concourse import tile
from concourse.bass_interp import get_hw_module
from concourse.bass_utils import run_bass_kernel_spmd
from concourse.mybir import ActivationFunctionType as Act

# ---------------------------------------------------------------- sizes
S, B, C, H, W = 10, 4, 19, 512, 512
HW = H * W
NCORES = 8
SLAB = HW // NCORES          # pixels per (core, b) = 32768
G = 6                        # chunks per region (G*C = 114 partitions)
FS = [2048, 2048, 1366]      # pixels per chunk, per region
OFFS = [0, 12288, 24576]     # region pixel offsets (6*F each)
PAD = 4                      # 24576 + 6*1366 = 32772 = SLAB + 4
MM_N = 512                   # matmul free-dim chunk (PSUM bank limit)
F32 = mybir.dt.float32
BF16 = mybir.dt.bfloat16


def _combined_act_tables():
    """Restrict ACT table selection to natural_log_exp_and_others so the
    interleaved exp/ln activations share ONE table set (the default pass
    alternates exp_and_others/natural_log -> 24 x 2.7us reloads)."""
    import concourse.hw_specs as hw_specs

    orig = hw_specs.get_activation_tables

    def patched(arch):
        t = orig(arch)
        if "natural_log_exp_and_others" not in t:
            return t
        return {
            name: (funcs if name == "natural_log_exp_and_others" else set())
            for name, funcs in t.items()
        }

    return orig, patched


def build_program():
    import concourse.bacc as bacc_mod

    orig, patched = _combined_act_tables()
    bacc_mod.get_activation_tables = patched
    try:
        return _build_program_inner()
    finally:
        bacc_mod.get_activation_tables = orig


def _build_program_inner():
    nc = bacc.Bacc("TRN2", target_bir_lowering=False, debug=False,
                   num_devices=NCORES)

    # eps_r{k}[b, j, c, s, x]: per-partition (j,c) line holds all 10
    # samples' pixels contiguously; one DMA per sample-pair -> 2*F lines
    eps_h = [
        nc.dram_tensor(f"eps_r{k}", [B, G, C, S, FS[k]], BF16,
                       kind="ExternalInput")
        for k in range(3)
    ]
    # mlv_b[b, p, :]: per-partition row = [mean r0|r1|r2, logvar r0|r1|r2]
    FTOT = sum(FS)
    mlv_h = nc.dram_tensor("mlv_b", [B, G * C, 2 * FTOT], BF16,
                           kind="ExternalInput")
    # label-selected views, packed [128, ...] with p = b*32 + x//1024
    eg_h = nc.dram_tensor("eg_il", [128, S, 1024], BF16,
                          kind="ExternalInput")
    mlvg_h = nc.dram_tensor("mlvg_il", [128, 2, 1024], BF16,
                            kind="ExternalInput")
    sel6_h = nc.dram_tensor("sel6", [S, 114, 64], BF16, kind="ExternalInput")
    lse_h = nc.dram_tensor("lse_out", [60, 1], F32, kind="ExternalOutput")
    lab_o_h = nc.dram_tensor("lab_out", [128, 1], F32, kind="ExternalOutput")

    with tile.TileContext(nc) as tc:
        with (
            tc.tile_pool(name="consts", bufs=1) as consts,
            tc.tile_pool(name="epsp", bufs=6) as eps_pool,
            tc.tile_pool(name="region", bufs=2) as region_pool,
            tc.tile_pool(name="work", bufs=3) as work_pool,
            tc.tile_pool(name="accp", bufs=1) as acc_pool,
            tc.tile_pool(name="psum", bufs=2, space="PSUM") as psum_pool,
        ):
            sel_sb = []
            for s in range(S):
                t6 = consts.tile([114, 64], BF16, tag=f"sel6_{s}",
                                 name=f"sel6_{s}")
                nc.sync.dma_start(out=t6, in_=sel6_h.ap()[s])
                sel_sb.append(t6)

            acc_lse = acc_pool.tile([60, 1], F32)
            nc.vector.memset(acc_lse, 0.0)

            # ---------------- main loop over (image, region)
            for b in range(B):
                # whole-image mean/log_var up front: one DMA + one std
                # activation, so region boundaries never wait on ACT
                mlvt = region_pool.tile([114, 2 * FTOT], BF16, tag="mlv")
                moff = b * G * C * 2 * FTOT
                nc.scalar.dma_start(
                    out=mlvt[:112, :],
                    in_=bass.AP(tensor=mlv_h, offset=moff,
                                ap=[[2 * FTOT, 112], [1, 2 * FTOT]]),
                )
                nc.scalar.dma_start(
                    out=mlvt[112:114, :],
                    in_=bass.AP(tensor=mlv_h,
                                offset=moff + 112 * 2 * FTOT,
                                ap=[[2 * FTOT, 2], [1, 2 * FTOT]]),
                )
                stdb = region_pool.tile([114, FTOT], BF16, tag="std")
                nc.scalar.activation(stdb, mlvt[:, FTOT:2 * FTOT],
                                     Act.Exp, scale=0.5)

                for r in range(3):
                    f = FS[r]
                    fo = sum(FS[:r])
                    mt = mlvt[:, fo:fo + f]
                    std = stdb[:, fo:fo + f]
                    # one DMA per sample-pair, alternating HWDGE rings so
                    # lines spread across more SDMA engines
                    epts = []
                    for sp in range(S // 2):
                        et = eps_pool.tile([114, 2 * FS[0]], BF16,
                                           tag="ept")
                        eng = nc.sync if sp % 2 == 0 else nc.scalar
                        # the DGE splits a DMA's lines evenly across the
                        # largest SDMA-engine count dividing the partition
                        # count: 114 -> 6 engines, 112 -> all 16. Issue as
                        # 112 + 2 partitions to engage the full pool.
                        off = b * G * C * S * f + 2 * sp * f
                        eng.dma_start(
                            out=et[:112, :2 * f],
                            in_=bass.AP(
                                tensor=eps_h[r],
                                offset=off,
                                ap=[[S * f, 112], [1, 2 * f]],
                            ),
                        )
                        eng.dma_start(
                            out=et[112:114, :2 * f],
                            in_=bass.AP(
                                tensor=eps_h[r],
                                offset=off + 112 * S * f,
                                ap=[[S * f, 2], [1, 2 * f]],
                            ),
                        )
                        epts.append(et)
                    psum_t = psum_pool.tile([64, FS[0]], F32, tag="psum")

                    for sp in range(S // 2):
                        t2 = work_pool.tile([114, 2 * FS[0]], BF16,
                                            tag="t2p")
                        for h in range(2):
                            half = t2[:, h * f:(h + 1) * f]
                            nc.vector.tensor_mul(
                                half, epts[sp][:, h * f:(h + 1) * f],
                                std,
                            )
                            nc.vector.tensor_add(half, half, mt)
                        e1 = work_pool.tile([114, 2 * FS[0]], BF16,
                                            tag="e1p")
                        nc.scalar.activation(e1[:, :2 * f], t2[:, :2 * f],
                                             Act.Exp)
                        # class-sum matmuls: free-dim chunks of <=512
                        nk = (f + MM_N - 1) // MM_N
                        for h in range(2):
                            s = 2 * sp + h
                            for k in range(nk):
                                c0 = k * MM_N
                                c1 = min(f, c0 + MM_N)
                                nc.tensor.matmul(
                                    psum_t[:, c0:c1],
                                    sel_sb[s],
                                    e1[:, h * f + c0:h * f + c1],
                                    start=(sp == 0 and h == 0),
                                    stop=(sp == S // 2 - 1 and h == 1),
                                )

                    # ln directly from PSUM; accum_out sums over pixels
                    lnb = work_pool.tile([64, FS[0]], BF16, tag="lnb")
                    lse_p = work_pool.tile([60, 1], F32, tag="lsep")
                    nc.scalar.activation(lnb[:60, :f], psum_t[:60, :f],
                                         Act.Ln, accum_out=lse_p)
                    nc.vector.tensor_add(acc_lse, acc_lse, lse_p)

            # ---------------- label side: host-staged gathered views
            # (emitted last so its DMAs/DVE ops fill late-program slack
            # instead of blocking the prologue)
            egt = consts.tile([128, S * 1024], BF16, tag="egt")
            nc.scalar.dma_start(out=egt, in_=bass.AP(
                tensor=eg_h, offset=0, ap=[[S * 1024, 128], [1, S * 1024]]))
            mlvgt = consts.tile([128, 2 * 1024], BF16, tag="mlvgt")
            nc.scalar.dma_start(out=mlvgt, in_=bass.AP(
                tensor=mlvg_h, offset=0, ap=[[2 * 1024, 128], [1, 2 * 1024]]))
            stdg = consts.tile([128, 1024], BF16, tag="stdg")
            nc.scalar.activation(stdg, mlvgt[:, 1024:2048], Act.Exp,
                                 scale=0.5)
            egs = consts.tile([128, 1024], BF16, tag="egs")
            nc.vector.tensor_add(egs, egt[:, 0:1024], egt[:, 1024:2048])
            for s in range(2, S):
                nc.vector.tensor_add(
                    egs, egs, egt[:, s * 1024:(s + 1) * 1024])
            lgt = consts.tile([128, 1024], BF16, tag="lgt")
            nc.vector.tensor_mul(lgt, egs, stdg)
            lab_p = acc_pool.tile([128, 1], F32)
            lgu = consts.tile([128, 1024], BF16, tag="lgu")
            # lgu = 10*mean_g + std_g*eps_sum_g, summed over pixels
            nc.vector.scalar_tensor_tensor(
                lgu, mlvgt[:, 0:1024], 10.0, lgt,
                mybir.AluOpType.mult, mybir.AluOpType.add,
                accum_out=lab_p,
            )
            nc.sync.dma_start(out=lab_o_h.ap(), in_=lab_p)

            nc.sync.dma_start(out=lse_h.ap(), in_=acc_lse)

    nc.compile()
    nc.m = get_hw_module(nc.m)
    return nc


def _sels():
    # partition p = j * 19 + c  (chunk-outer, class-inner)
    sel6 = np.zeros((S, 114, 64), dtype=ml_dtypes.bfloat16)
    for s in range(S):
        for p in range(114):
            sel6[s, p, 6 * s + p // C] = 1.0
    return sel6


def kernel(mean, log_var, label, eps, _trace=False):
    mean = np.asarray(mean, dtype=np.float32).reshape(B, C, HW)
    log_var = np.asarray(log_var, dtype=np.float32).reshape(B, C, HW)
    label_i = np.asarray(label).reshape(B, HW).astype(np.int64)
    eps_r = np.asarray(eps, dtype=np.float32).reshape(S, B, C, HW)

    # label-gathered views (index staging; arithmetic stays on device)
    bi = np.arange(B)[:, None]
    ci = np.arange(HW)[None, :]
    mg = mean[bi, label_i, ci]                               # [B, HW]
    lvg = log_var[bi, label_i, ci]                           # [B, HW]
    eg = eps_r[:, bi, label_i, ci]                           # [S, B, HW]

    sel6 = _sels()
    FTOT = sum(FS)
    in_maps = []
    for c in range(NCORES):
        lo, hi = c * SLAB, (c + 1) * SLAB
        im = {"sel6": sel6}
        mparts, lparts = [], []
        for k in range(3):
            f = FS[k]
            a, z = lo + OFFS[k], lo + OFFS[k] + G * f
            if z <= hi:
                esl = eps_r[:, :, :, a:z]
                msl = mean[:, :, a:z]
                lsl = log_var[:, :, a:z]
            else:  # pad the 4-pixel tail of region 2 with zeros
                pw = z - hi
                esl = np.concatenate(
                    [eps_r[:, :, :, a:hi],
                     np.zeros((S, B, C, pw), np.float32)], axis=3)
                msl = np.concatenate(
                    [mean[:, :, a:hi], np.zeros((B, C, pw), np.float32)],
                    axis=2)
                lsl = np.concatenate(
                    [log_var[:, :, a:hi], np.zeros((B, C, pw), np.float32)],
                    axis=2)
            im[f"eps_r{k}"] = np.ascontiguousarray(
                esl.reshape(S, B, C, G, f).transpose(1, 3, 2, 0, 4)
            ).astype(ml_dtypes.bfloat16)
            # [B, C, G, f] -> [B, p=(j,c), f]
            mparts.append(msl.reshape(B, C, G, f).transpose(0, 2, 1, 3)
                          .reshape(B, G * C, f))
            lparts.append(lsl.reshape(B, C, G, f).transpose(0, 2, 1, 3)
                          .reshape(B, G * C, f))
        im["mlv_b"] = np.ascontiguousarray(
            np.concatenate(mparts + lparts, axis=2)
        ).astype(ml_dtypes.bfloat16)
        im["eg_il"] = np.ascontiguousarray(
            eg[:, :, lo:hi].reshape(S, B * 32, 1024).transpose(1, 0, 2)
        ).astype(ml_dtypes.bfloat16)
        im["mlvg_il"] = np.ascontiguousarray(
            np.stack([mg[:, lo:hi], lvg[:, lo:hi]], axis=1)
            .reshape(B, 2, 32, 1024).transpose(0, 2, 1, 3)
            .reshape(128, 2, 1024)).astype(ml_dtypes.bfloat16)
        in_maps.append(im)

    nc = build_program()
    res = run_bass_kernel_spmd(
        nc, in_maps, core_ids=list(range(NCORES)), trace=_trace
    )
    global last_results
    last_results = res

    total = np.float64(0.0)
    for c in range(NCORES):
        total += res.results[c]["lse_out"].astype(np.float64).sum()
        total -= res.results[c]["lab_out"].astype(np.float64).sum()
    # remove the analytic contribution of the PAD pixels:
    # each pad pixel adds ln(19) per (core, image, sample)
    total -= np.float64(NCORES * B * S * PAD) * np.log(np.float64(C))
    loss = total / float(S * B * HW)
    return np.float32(loss)


# revision 25
# speedup vs baseline: 1.2578x; 1.0030x over previous
"""Trainium2 Bass kernel for MC-sampled cross-entropy-with-variance loss.

Computes mean over (s, b, h, w) of
    nll = logsumexp_c(mean + exp(0.5*log_var)*eps[s]) - logit[label]
distributed over 8 NeuronCores by sharding the H*W pixel axis.

v3 layout: every region is a FULL 6-chunk pack (19 classes x 6 chunks =
114 partitions); region pixel widths are (2048, 2048, 1366) so the
16384+8196-pixel slab is covered with only 4 pad pixels per image
instead of a 4-chunk tail region idling a third of the partitions.
The pad contributes exactly ln(19) per (sample, pad pixel) and is
subtracted analytically on the host. eps is host-interleaved to
[b, chunk, class, sample, pixel] so each (image, region, sample-pair)
is one DMA with 2*F-byte-contiguous lines, alternating the two HWDGE
rings; mean/log_var ride one merged DMA per region. The label side
uses host-staged index-selected views (pure gather; all arithmetic on
device). ln runs directly on the PSUM sumexp with accum_out, and exp/ln
share the natural_log_exp_and_others ACT table set (one load).
"""

import numpy as np
import ml_dtypes

import concourse.bass as bass
import concourse.bacc as bacc
import concourse.mybir as mybir
from concourse import tile
from concourse.bass_interp import get_hw_module
from concourse.bass_utils import run_bass_kernel_spmd
from concourse.mybir import ActivationFunctionType as Act

# ---------------------------------------------------------------- sizes
S, B, C, H, W = 10, 4, 19, 512, 512
HW = H * W
NCORES = 8
SLAB = HW // NCORES          # pixels per (core, b) = 32768
G = 6                        # chunks per region (G*C = 114 partitions)
FS = [2048, 2048, 1366]      # pixels per chunk, per region
OFFS = [0, 12288, 24576]     # region pixel offsets (6*F each)
PAD = 4                      # 24576 + 6*1366 = 32772 = SLAB + 4
MM_N = 512                   # matmul free-dim chunk (PSUM bank limit)
F32 = mybir.dt.float32
BF16 = mybir.dt.bfloat16


def _combined_act_tables():
    """Restrict ACT table selection to natural_log_exp_and_others so the
    interleaved exp/ln activations share ONE table set (the default pass
    alternates exp_and_others/natural_log -> 24 x 2.7us reloads)."""
    import concourse.hw_specs as hw_specs

    orig = hw_specs.get_activation_tables

    def patched(arch):
        t = orig(arch)
        if "natural_log_exp_and_others" not in t:
            return t
        return {
            name: (funcs if name == "natural_log_exp_and_others" else set())
            for name, funcs in t.items()
        }

    return orig, patched


def build_program():
    import concourse.bacc as bacc_mod

    orig, patched = _combined_act_tables()
    bacc_mod.get_activation_tables = patched
    try:
        return _build_program_inner()
    finally:
        bacc_mod.get_activation_tables = orig


def _build_program_inner():
    nc = bacc.Bacc("TRN2", target_bir_lowering=False, debug=False,
                   num_devices=NCORES)

    # eps_r{k}[b, j, c, s, x]: per-partition (j,c) line holds all 10
    # samples' pixels contiguously; one DMA per sample-pair -> 2*F lines
    eps_h = [
        nc.dram_tensor(f"eps_r{k}", [B, G, C, S, FS[k]], BF16,
                       kind="ExternalInput")
        for k in range(3)
    ]
    # mlv_b[b, p, :]: per-partition row = [mean r0|r1|r2, logvar r0|r1|r2]
    FTOT = sum(FS)
    mlv_h = nc.dram_tensor("mlv_b", [B, G * C, 2 * FTOT], BF16,
                           kind="ExternalInput")
    # label-selected views, packed [128, ...] with p = b*32 + x//1024
    eg_h = nc.dram_tensor("eg_il", [128, S, 1024], BF16,
                          kind="ExternalInput")
    mlvg_h = nc.dram_tensor("mlvg_il", [128, 2, 1024], BF16,
                            kind="ExternalInput")
    # sel6 transposed to [114, S*64] so all selectors ride ONE DMA
    sel6_h = nc.dram_tensor("sel6", [114, S * 64], BF16,
                            kind="ExternalInput")
    lse_h = nc.dram_tensor("lse_out", [B * 3, 60, 1], F32,
                           kind="ExternalOutput")
    lab_o_h = nc.dram_tensor("lab_out", [128, 1], F32, kind="ExternalOutput")

    with tile.TileContext(nc) as tc:
        with (
            tc.tile_pool(name="consts", bufs=1) as consts,
            tc.tile_pool(name="epsp", bufs=6) as eps_pool,
            tc.tile_pool(name="region", bufs=2) as region_pool,
            tc.tile_pool(name="work", bufs=3) as work_pool,
            tc.tile_pool(name="accp", bufs=1) as acc_pool,
            tc.tile_pool(name="psum", bufs=2, space="PSUM") as psum_pool,
        ):
            sel_all = consts.tile([114, S * 64], BF16, tag="sel6")
            nc.sync.dma_start(out=sel_all, in_=sel6_h.ap())
            sel_sb = [sel_all[:, s * 64:(s + 1) * 64] for s in range(S)]

            # mlv layout per image row: [logvar FTOT | mean FTOT]; the
            # logvar half is DMA'd first so std never waits on the mean
            def load_mlv(b):
                mlvt = region_pool.tile([114, 2 * FTOT], BF16, tag="mlv")
                moff = b * G * C * 2 * FTOT
                for c0, c1 in ((0, FTOT), (FTOT, 2 * FTOT)):
                    nc.scalar.dma_start(
                        out=mlvt[:112, c0:c1],
                        in_=bass.AP(tensor=mlv_h, offset=moff + c0,
                                    ap=[[2 * FTOT, 112], [1, FTOT]]),
                    )
                    nc.scalar.dma_start(
                        out=mlvt[112:114, c0:c1],
                        in_=bass.AP(tensor=mlv_h,
                                    offset=moff + 112 * 2 * FTOT + c0,
                                    ap=[[2 * FTOT, 2], [1, FTOT]]),
                    )
                stdb = region_pool.tile([114, FTOT], BF16, tag="std")
                nc.scalar.activation(stdb, mlvt[:, 0:FTOT], Act.Exp,
                                     scale=0.5)
                return mlvt, stdb

            # ---------------- main loop over (image, region)
            mlv_std = load_mlv(0)
            pending_ln = None   # (psum_t, f, out_idx) of previous region
            for b in range(B):
                mlvt, stdb = mlv_std
                for r in range(3):
                    f = FS[r]
                    fo = sum(FS[:r])
                    mt = mlvt[:, FTOT + fo:FTOT + fo + f]
                    std = stdb[:, fo:fo + f]
                    # one DMA per sample-pair, alternating HWDGE rings so
                    # lines spread across more SDMA engines
                    epts = []
                    for sp in range(S // 2):
                        et = eps_pool.tile([114, 2 * FS[0]], BF16,
                                           tag="ept")
                        eng = nc.sync if sp % 2 == 0 else nc.scalar
                        # the DGE splits a DMA's lines evenly across the
                        # largest SDMA-engine count dividing the partition
                        # count: 114 -> 6 engines, 112 -> all 16. Issue as
                        # 112 + 2 partitions to engage the full pool.
                        off = b * G * C * S * f + 2 * sp * f
                        eng.dma_start(
                            out=et[:112, :2 * f],
                            in_=bass.AP(
                                tensor=eps_h[r],
                                offset=off,
                                ap=[[S * f, 112], [1, 2 * f]],
                            ),
                        )
                        eng.dma_start(
                            out=et[112:114, :2 * f],
                            in_=bass.AP(
                                tensor=eps_h[r],
                                offset=off + 112 * S * f,
                                ap=[[S * f, 2], [1, 2 * f]],
                            ),
                        )
                        epts.append(et)
                    psum_t = psum_pool.tile([64, FS[0]], F32, tag="psum")

                    for sp in range(S // 2):
                        t2 = work_pool.tile([114, 2 * FS[0]], BF16,
                                            tag="t2p")
                        for h in range(2):
                            half = t2[:, h * f:(h + 1) * f]
                            nc.vector.tensor_mul(
                                half, epts[sp][:, h * f:(h + 1) * f],
                                std,
                            )
                            nc.vector.tensor_add(half, half, mt)
                        e1 = work_pool.tile([114, 2 * FS[0]], BF16,
                                            tag="e1p")
                        nc.scalar.activation(e1[:, :2 * f], t2[:, :2 * f],
                                             Act.Exp)
                        # class-sum matmuls: free-dim chunks of <=512
                        nk = (f + MM_N - 1) // MM_N
                        for h in range(2):
                            s = 2 * sp + h
                            for k in range(nk):
                                c0 = k * MM_N
                                c1 = min(f, c0 + MM_N)
                                nc.tensor.matmul(
                                    psum_t[:, c0:c1],
                                    sel_sb[s],
                                    e1[:, h * f + c0:h * f + c1],
                                    start=(sp == 0 and h == 0),
                                    stop=(sp == S // 2 - 1 and h == 1),
                                )

                    # software-pipeline next image's mean/logvar + std so
                    # image boundaries never stall on ACT
                    if r == 0 and b + 1 < B:
                        mlv_std = load_mlv(b + 1)

                    # ln of the PREVIOUS region's psum, emitted after this
                    # region's exps so it never blocks the ACT queue; its
                    # per-region sum goes straight to DRAM (no serial
                    # accumulator on the DVE critical path)
                    def emit_ln(p_psum, p_f, p_idx):
                        lnb = work_pool.tile([64, FS[0]], BF16, tag="lnb")
                        lse_p = work_pool.tile([60, 1], F32, tag="lsep")
                        nc.scalar.activation(lnb[:60, :p_f],
                                             p_psum[:60, :p_f],
                                             Act.Ln, accum_out=lse_p)
                        nc.sync.dma_start(out=lse_h.ap()[p_idx],
                                          in_=lse_p)
                    if pending_ln is not None:
                        emit_ln(*pending_ln)
                    pending_ln = (psum_t, f, 3 * b + r)

            emit_ln(*pending_ln)

            # ---------------- label side: host-staged gathered views
            # (emitted last so its DMAs/DVE ops fill late-program slack
            # instead of blocking the prologue)
            egt = consts.tile([128, S * 1024], BF16, tag="egt")
            nc.scalar.dma_start(out=egt, in_=bass.AP(
                tensor=eg_h, offset=0, ap=[[S * 1024, 128], [1, S * 1024]]))
            mlvgt = consts.tile([128, 2 * 1024], BF16, tag="mlvgt")
            nc.scalar.dma_start(out=mlvgt, in_=bass.AP(
                tensor=mlvg_h, offset=0, ap=[[2 * 1024, 128], [1, 2 * 1024]]))
            stdg = consts.tile([128, 1024], BF16, tag="stdg")
            nc.scalar.activation(stdg, mlvgt[:, 1024:2048], Act.Exp,
                                 scale=0.5)
            egs = consts.tile([128, 1024], BF16, tag="egs")
            nc.vector.tensor_add(egs, egt[:, 0:1024], egt[:, 1024:2048])
            for s in range(2, S):
                nc.vector.tensor_add(
                    egs, egs, egt[:, s * 1024:(s + 1) * 1024])
            lgt = consts.tile([128, 1024], BF16, tag="lgt")
            nc.vector.tensor_mul(lgt, egs, stdg)
            lab_p = acc_pool.tile([128, 1], F32)
            lgu = consts.tile([128, 1024], BF16, tag="lgu")
            # lgu = 10*mean_g + std_g*eps_sum_g, summed over pixels
            nc.vector.scalar_tensor_tensor(
                lgu, mlvgt[:, 0:1024], 10.0, lgt,
                mybir.AluOpType.mult, mybir.AluOpType.add,
                accum_out=lab_p,
            )
            nc.sync.dma_start(out=lab_o_h.ap(), in_=lab_p)

    nc.compile()
    nc.m = get_hw_module(nc.m)
    return nc


def _sels():
    # partition p = j * 19 + c  (chunk-outer, class-inner);
    # [114, S*64] so all 10 selectors ride one DMA
    sel6 = np.zeros((114, S, 64), dtype=ml_dtypes.bfloat16)
    for s in range(S):
        for p in range(114):
            sel6[p, s, 6 * s + p // C] = 1.0
    return sel6.reshape(114, S * 64)


def kernel(mean, log_var, label, eps, _trace=False):
    mean = np.asarray(mean, dtype=np.float32).reshape(B, C, HW)
    log_var = np.asarray(log_var, dtype=np.float32).reshape(B, C, HW)
    label_i = np.asarray(label).reshape(B, HW).astype(np.int64)
    eps_r = np.asarray(eps, dtype=np.float32).reshape(S, B, C, HW)

    # label-gathered views (index staging; arithmetic stays on device)
    bi = np.arange(B)[:, None]
    ci = np.arange(HW)[None, :]
    mg = mean[bi, label_i, ci]                               # [B, HW]
    lvg = log_var[bi, label_i, ci]                           # [B, HW]
    eg = eps_r[:, bi, label_i, ci]                           # [S, B, HW]

    sel6 = _sels()
    FTOT = sum(FS)
    in_maps = []
    for c in range(NCORES):
        lo, hi = c * SLAB, (c + 1) * SLAB
        im = {"sel6": sel6}
        mparts, lparts = [], []
        for k in range(3):
            f = FS[k]
            a, z = lo + OFFS[k], lo + OFFS[k] + G * f
            if z <= hi:
                esl = eps_r[:, :, :, a:z]
                msl = mean[:, :, a:z]
                lsl = log_var[:, :, a:z]
            else:  # pad the 4-pixel tail of region 2 with zeros
                pw = z - hi
                esl = np.concatenate(
                    [eps_r[:, :, :, a:hi],
                     np.zeros((S, B, C, pw), np.float32)], axis=3)
                msl = np.concatenate(
                    [mean[:, :, a:hi], np.zeros((B, C, pw), np.float32)],
                    axis=2)
                lsl = np.concatenate(
                    [log_var[:, :, a:hi], np.zeros((B, C, pw), np.float32)],
                    axis=2)
            im[f"eps_r{k}"] = np.ascontiguousarray(
                esl.reshape(S, B, C, G, f).transpose(1, 3, 2, 0, 4)
            ).astype(ml_dtypes.bfloat16)
            # [B, C, G, f] -> [B, p=(j,c), f]
            mparts.append(msl.reshape(B, C, G, f).transpose(0, 2, 1, 3)
                          .reshape(B, G * C, f))
            lparts.append(lsl.reshape(B, C, G, f).transpose(0, 2, 1, 3)
                          .reshape(B, G * C, f))
        im["mlv_b"] = np.ascontiguousarray(
            np.concatenate(mparts + lparts, axis=2)
        ).astype(ml_dtypes.bfloat16)
        im["eg_il"] = np.ascontiguousarray(
            eg[:, :, lo:hi].reshape(S, B * 32, 1024).transpose(1, 0, 2)
        ).astype(ml_dtypes.bfloat16)
        im["mlvg_il"] = np.ascontiguousarray(
            np.stack([mg[:, lo:hi], lvg[:, lo:hi]], axis=1)
            .reshape(B, 2, 32, 1024).transpose(0, 2, 1, 3)
            .reshape(128, 2, 1024)).astype(ml_dtypes.bfloat16)
        in_maps.append(im)

    nc = build_program()
    res = run_bass_kernel_spmd(
        nc, in_maps, core_ids=list(range(NCORES)), trace=_trace
    )
    global last_results
    last_results = res

    total = np.float64(0.0)
    for c in range(NCORES):
        total += res.results[c]["lse_out"].astype(np.float64).sum()
        total -= res.results[c]["lab_out"].astype(np.float64).sum()
    # remove the analytic contribution of the PAD pixels:
    # each pad pixel adds ln(19) per (core, image, sample)
    total -= np.float64(NCORES * B * S * PAD) * np.log(np.float64(C))
    loss = total / float(S * B * HW)
    return np.float32(loss)
